# revision 19
# baseline (speedup 1.0000x reference)
"""Trainium2 Bass kernel for MessagePassingUnitGatingWithRelnessLogits.

Computation (per row n of N=32768):
  paired = relu(concat(unary, pair))                       # [2048]
  h      = relu(LN(paired) * gamma + beta)                 # [2048]
  gate   = mean_j sigmoid(h @ W + b)                       # scalar
  ha     = relu(LN(aux) * gamma_a + beta_a)                # [256]
  agate  = mean_j sigmoid(ha @ W_aux + b_aux)              # scalar
  logit  = gw * rev_sigmoid(gate) + agw * rev_sigmoid(agate)
  g      = sigmoid(logit) * auxiliary_gating_weight
  out    = pair * g
Returns (out [N,1024] f32, g [N] f32).

Sharding: pure data-parallel over N across 8 cores (4096 rows each).

Architecture notes:
 - All transcendentals via exp/ln only -> single ACT table set, no swaps.
 - Fast path (beta==0, gamma>0): relu((x-mu)*rs*gamma) = rs * gamma*relu(x-mu),
   so gamma folds into W on the host and the per-row rs folds into the tiny
   [128,72] PSUM evacuation.  Normalize+relu is ONE dual-op DVE pass.
 - LN stats via chunked bn_stats/bn_aggr (no accumulator opcodes needed).
 - h transposed via one DMA xbar transpose per tile (no PE transposes).
 - Gate matmul: lhsT = hT chunk (stationary), rhs = W chunk (moving),
   y lands row-major [128 rows, 72] in PSUM; gate means are DVE free-dim
   reductions; per-row scalar chains batched over groups of 8 tiles.
"""

import os
import sys

import numpy as np

if "/opt/trn_rl_repo" not in sys.path:
    sys.path.insert(0, "/opt/trn_rl_repo")

import ml_dtypes  # noqa: E402

import bass_rust as _bass_rust  # noqa: E402

import concourse.bacc as bacc  # noqa: E402
import concourse.bass as bass  # noqa: E402
import concourse.mybir as mybir  # noqa: E402
import concourse.tile as tile  # noqa: E402
from concourse.bass import ts  # noqa: E402
from concourse.bass_utils import run_bass_kernel_spmd  # noqa: E402
from concourse.hw_specs import get_activation_tables  # noqa: E402

F32 = mybir.dt.float32
BF16 = mybir.dt.bfloat16
ALU = mybir.AluOpType
ACT_F = mybir.ActivationFunctionType

N_CORES = 8
N = 32768
D = 1024
D2 = 2048
A = 256
FD = 64
F8 = 8
FT = FD + F8                 # 72 packed gate columns (main | aux)
EPS = 1e-5

ROWS = N // N_CORES          # 4096 rows per core
P = 128                      # partition tile
NT = ROWS // P               # 32 tiles per core
G = 8                        # tiles per group (batched scalar chains)
NG = NT // G                 # 4 groups
KC = D2 // P                 # 16 contraction chunks (main)
KA = A // P                  # 2 contraction chunks (aux)
KT = KC + KA                 # 18 chunks in the fused transpose
BNC = 4                      # bn_stats chunks (512 each) for the 2048 dim

ACT_SET = "natural_log_exp_and_others"

# module-level stash so test.py can read profiling info
last_results = None


class _Bacc(bacc.Bacc):
    """Bacc that pins every ACTIVATE to one activation-table set.

    The default chooser picks the first set containing each function,
    which thrashes between sets (~2.7us per swap).  Every function this
    kernel uses (relu, exp, ln, copy, identity) lives in
    `natural_log_exp_and_others`, so blank out the other sets (keeping
    list positions, which are the set ids).
    """

    def insert_act_table_loads(self):
        has_activation = any(
            isinstance(i, mybir.InstActivation)
            for b in self.main_func.blocks
            for i in b.instructions
        )
        if not has_activation:
            return
        tables = [
            (name, (funcs if name == ACT_SET else set()))
            for name, funcs in get_activation_tables(self.m.arch).items()
        ]
        _bass_rust.insert_act_table_loads(self, tables)


def _build_nc(gw: float, agw_w: float, fast_main: bool, fast_aux: bool):
    nc = _Bacc("TRN2", target_bir_lowering=False, debug=False, num_devices=N_CORES)

    # ---------------- DRAM I/O ----------------
    du = nc.dram_tensor("u", [ROWS, D], F32, kind="ExternalInput")
    dp = nc.dram_tensor("p", [ROWS, D], F32, kind="ExternalInput")
    dax = nc.dram_tensor("ax", [ROWS, A], F32, kind="ExternalInput")
    dagw = nc.dram_tensor("agw_t", [P, NT], F32, kind="ExternalInput")
    dwr = nc.dram_tensor("wr", [P, KC * FD], BF16, kind="ExternalInput")
    dwaux = nc.dram_tensor("waux_r", [P, KA * F8], BF16, kind="ExternalInput")
    db = nc.dram_tensor("b_row", [1, FT], F32, kind="ExternalInput")
    dgb = dnbb = dgba = dnbba = None
    if not fast_main:
        dgb = nc.dram_tensor("gam_row", [1, D2], F32, kind="ExternalInput")
        dnbb = nc.dram_tensor("negbeta_row", [1, D2], F32, kind="ExternalInput")
    if not fast_aux:
        dgba = nc.dram_tensor("gam_aux_row", [1, A], F32, kind="ExternalInput")
        dnbba = nc.dram_tensor("negbeta_aux_row", [1, A], F32, kind="ExternalInput")

    dout = nc.dram_tensor("out", [ROWS, D], F32, kind="ExternalOutput")
    dg = nc.dram_tensor("g_t", [P, NT], F32, kind="ExternalOutput")

    import contextlib

    ctx = contextlib.ExitStack()
    with ctx:
        tc = ctx.enter_context(tile.TileContext(nc))

        consts = ctx.enter_context(tc.tile_pool(name="consts", bufs=1))
        io_u = ctx.enter_context(tc.tile_pool(name="io_u", bufs=3))
        io_p = ctx.enter_context(tc.tile_pool(name="io_p", bufs=G + 2))
        io_ax = ctx.enter_context(tc.tile_pool(name="io_ax", bufs=G + 2))
        prd = ctx.enter_context(tc.tile_pool(name="prd", bufs=G + 2))
        hp = ctx.enter_context(tc.tile_pool(name="hp", bufs=3))
        htp = ctx.enter_context(tc.tile_pool(name="htp", bufs=3))
        stg = ctx.enter_context(tc.tile_pool(name="stg", bufs=2))
        outp = ctx.enter_context(tc.tile_pool(name="outp", bufs=3))
        small = ctx.enter_context(tc.tile_pool(name="small", bufs=3))
        ps_y = ctx.enter_context(tc.tile_pool(name="ps_y", bufs=6, space="PSUM"))

        # ---------------- constants ----------------
        wr_sb = consts.tile([P, KC, FD], BF16)
        nc.sync.dma_start(out=wr_sb, in_=dwr[:, :].rearrange("p (k f) -> p k f", k=KC))
        waux_sb = consts.tile([P, KA, F8], BF16)
        nc.sync.dma_start(
            out=waux_sb, in_=dwaux[:, :].rearrange("p (k f) -> p k f", k=KA)
        )
        agw_sb = consts.tile([P, NT], F32)
        nc.sync.dma_start(out=agw_sb, in_=dagw[:, :])

        def bcast_load(dram_ap, width, dt=F32):
            t = consts.tile([P, width], dt)
            src = bass.AP(tensor=dram_ap.tensor, offset=0, ap=[[0, P], [1, width]])
            nc.gpsimd.dma_start(out=t, in_=src)
            return t

        b_bc = bcast_load(db[:, :], FT)
        gb_b = nbb_b = gba_b = nbba_b = None
        if not fast_main:
            gb_b = bcast_load(dgb[:, :], D2, BF16)
            nbb_b = bcast_load(dnbb[:, :], D2, BF16)
        if not fast_aux:
            gba_b = bcast_load(dgba[:, :], A, BF16)
            nbba_b = bcast_load(dnbba[:, :], A, BF16)

        eps_col = consts.tile([P, 1], F32)
        nc.vector.memset(eps_col, EPS)
        g_staged = consts.tile([P, NT], F32)

        # ---------------- main loop ----------------
        for grp in range(NG):
            t0 = grp * G

            mvm = small.tile([P, G, 2], F32, tag="mvm")
            mva = small.tile([P, G, 2], F32, tag="mva")

            paired_l = []
            p_l = []
            ax_l = []
            # ---- phase A: load, relu, bn stats ----
            for i in range(G):
                t = t0 + i
                u_sb = io_u.tile([P, D], F32, tag="u")
                p_sb = io_p.tile([P, D], F32, tag="p")
                ax_sb = io_ax.tile([P, A], F32, tag="ax")
                nc.sync.dma_start(out=u_sb, in_=du[ts(t, P), :])
                nc.sync.dma_start(out=p_sb, in_=dp[ts(t, P), :])
                nc.sync.dma_start(out=ax_sb, in_=dax[ts(t, P), :])

                paired = prd.tile([P, D2], BF16, tag="paired")
                nc.scalar.activation(out=paired[:, 0:D], in_=u_sb, func=ACT_F.Relu)
                nc.scalar.activation(out=paired[:, D:D2], in_=p_sb, func=ACT_F.Relu)

                bnst = small.tile([P, BNC, 6], F32, tag="bnst")
                pv = paired[:, :].rearrange("p (c q) -> p c q", c=BNC)
                for c in range(BNC):
                    nc.vector.bn_stats(out=bnst[:, c, :], in_=pv[:, c, :])
                nc.vector.bn_aggr(out=mvm[:, i, :], in_=bnst)

                bnsta = small.tile([P, 6], F32, tag="bnsta")
                nc.vector.bn_stats(out=bnsta, in_=ax_sb)
                nc.vector.bn_aggr(out=mva[:, i, :], in_=bnsta)

                paired_l.append(paired)
                p_l.append(p_sb)
                ax_l.append(ax_sb)

            # ---- group stats: -mu and rs = exp(-0.5*ln(var+eps)) ----
            negmu8 = small.tile([P, G], F32, tag="negmu8")
            nc.vector.tensor_scalar(
                out=negmu8, in0=mvm[:, :, 0], scalar1=-1.0, scalar2=None, op0=ALU.mult
            )
            negmua8 = small.tile([P, G], F32, tag="negmua8")
            nc.vector.tensor_scalar(
                out=negmua8, in0=mva[:, :, 0], scalar1=-1.0, scalar2=None, op0=ALU.mult
            )
            var16 = small.tile([P, 2 * G], F32, tag="var16")
            nc.vector.tensor_copy(out=var16[:, 0:G], in_=mvm[:, :, 1])
            nc.vector.tensor_copy(out=var16[:, G : 2 * G], in_=mva[:, :, 1])
            lnv = small.tile([P, 2 * G], F32, tag="lnv")
            nc.scalar.activation(out=lnv, in_=var16, func=ACT_F.Ln, bias=eps_col)
            rs16 = small.tile([P, 2 * G], F32, tag="rs16")
            nc.scalar.activation(out=rs16, in_=lnv, func=ACT_F.Exp, scale=-0.5)

            # ---- phase B: normalize(+relu), xbar transpose, gate matmuls ----
            staged = stg.tile([P, G * FT], BF16, tag="staged")
            for i in range(G):
                hh = hp.tile([P, KT * P], BF16, tag="hh")
                if fast_main:
                    # r = relu(paired - mu); rs folded into PSUM evacuation
                    nc.vector.tensor_scalar(
                        out=hh[:, 0:D2], in0=paired_l[i],
                        scalar1=negmu8[:, i : i + 1], scalar2=0.0,
                        op0=ALU.add, op1=ALU.max,
                    )
                else:
                    # general: t = (x-mu)*rs; t *= gamma; t = max(t, -beta);
                    # (+beta folded into b via b_eff = b + beta @ W)
                    nc.vector.tensor_scalar(
                        out=hh[:, 0:D2], in0=paired_l[i],
                        scalar1=negmu8[:, i : i + 1], scalar2=rs16[:, i : i + 1],
                        op0=ALU.add, op1=ALU.mult,
                    )
                    nc.vector.scalar_tensor_tensor(
                        out=hh[:, 0:D2], in0=hh[:, 0:D2], scalar=0.0, in1=gb_b,
                        op0=ALU.bypass, op1=ALU.mult,
                    )
                    nc.vector.scalar_tensor_tensor(
                        out=hh[:, 0:D2], in0=hh[:, 0:D2], scalar=0.0, in1=nbb_b,
                        op0=ALU.bypass, op1=ALU.max,
                    )
                if fast_aux:
                    nc.vector.tensor_scalar(
                        out=hh[:, D2 : KT * P], in0=ax_l[i],
                        scalar1=negmua8[:, i : i + 1], scalar2=0.0,
                        op0=ALU.add, op1=ALU.max,
                    )
                else:
                    nc.vector.tensor_scalar(
                        out=hh[:, D2 : KT * P], in0=ax_l[i],
                        scalar1=negmua8[:, i : i + 1],
                        scalar2=rs16[:, G + i : G + i + 1],
                        op0=ALU.add, op1=ALU.mult,
                    )
                    nc.vector.scalar_tensor_tensor(
                        out=hh[:, D2 : KT * P], in0=hh[:, D2 : KT * P], scalar=0.0,
                        in1=gba_b, op0=ALU.bypass, op1=ALU.mult,
                    )
                    nc.vector.scalar_tensor_tensor(
                        out=hh[:, D2 : KT * P], in0=hh[:, D2 : KT * P], scalar=0.0,
                        in1=nbba_b, op0=ALU.bypass, op1=ALU.max,
                    )

                # one xbar transpose: [128, 2304] -> [128, 18, 128]
                hht = htp.tile([P, KT, P], BF16, tag="hht")
                nc.sync.dma_start_transpose(hht, hh)

                # gate matmuls: y[n, j] accumulates row-major in PSUM
                yps = ps_y.tile([P, FT], F32, tag="yps")
                for k in range(KC):
                    nc.tensor.matmul(
                        yps[:, 0:FD], lhsT=hht[:, k, :], rhs=wr_sb[:, k, :],
                        start=(k == 0), stop=(k == KC - 1),
                    )
                for k in range(KA):
                    nc.tensor.matmul(
                        yps[:, FD:FT], lhsT=hht[:, KC + k, :], rhs=waux_sb[:, k, :],
                        start=(k == 0), stop=(k == KA - 1),
                    )
                # evacuate with fused rs scale (fast path) + bias add
                sl = staged[:, i * FT : i * FT + FD]
                sla = staged[:, i * FT + FD : (i + 1) * FT]
                if fast_main:
                    nc.vector.scalar_tensor_tensor(
                        out=sl, in0=yps[:, 0:FD], scalar=rs16[:, i : i + 1],
                        in1=b_bc[:, 0:FD], op0=ALU.mult, op1=ALU.add,
                    )
                else:
                    nc.vector.scalar_tensor_tensor(
                        out=sl, in0=yps[:, 0:FD], scalar=0.0,
                        in1=b_bc[:, 0:FD], op0=ALU.bypass, op1=ALU.add,
                    )
                if fast_aux:
                    nc.vector.scalar_tensor_tensor(
                        out=sla, in0=yps[:, FD:FT], scalar=rs16[:, G + i : G + i + 1],
                        in1=b_bc[:, FD:FT], op0=ALU.mult, op1=ALU.add,
                    )
                else:
                    nc.vector.scalar_tensor_tensor(
                        out=sla, in0=yps[:, FD:FT], scalar=0.0,
                        in1=b_bc[:, FD:FT], op0=ALU.bypass, op1=ALU.add,
                    )

            # ---- phase C: batched sigmoid via exp/ln chain ----
            # sigma(z) = exp(-ln(1 + exp(-z)))
            ebuf = stg.tile([P, G * FT], BF16, tag="ebuf")
            nc.scalar.activation(out=ebuf, in_=staged, func=ACT_F.Exp, scale=-1.0)
            lbuf = stg.tile([P, G * FT], BF16, tag="lbuf")
            nc.scalar.activation(out=lbuf, in_=ebuf, func=ACT_F.Ln, bias=1.0)
            sig = stg.tile([P, G * FT], BF16, tag="sig")
            nc.scalar.activation(out=sig, in_=lbuf, func=ACT_F.Exp, scale=-1.0)

            # ---- phase D: means, logit chain, g, final mul ----
            sig3 = sig[:, :].rearrange("p (g f) -> p g f", g=G)
            m8 = small.tile([P, G], F32, tag="m8")
            nc.vector.tensor_reduce(
                out=m8, in_=sig3[:, :, 0:FD], axis=mybir.AxisListType.X, op=ALU.add
            )
            ma8 = small.tile([P, G], F32, tag="ma8")
            nc.vector.tensor_reduce(
                out=ma8, in_=sig3[:, :, FD:FT], axis=mybir.AxisListType.X, op=ALU.add
            )
            c16 = small.tile([P, 2 * G], F32, tag="c16")
            nc.vector.tensor_scalar(
                out=c16[:, 0:G], in0=m8, scalar1=1.0 / FD, scalar2=0.999,
                op0=ALU.mult, op1=ALU.min,
            )
            nc.vector.tensor_scalar(
                out=c16[:, G : 2 * G], in0=ma8, scalar1=1.0 / F8, scalar2=0.999,
                op0=ALU.mult, op1=ALU.min,
            )
            c16b = small.tile([P, 2 * G], F32, tag="c16b")
            nc.vector.tensor_scalar(
                out=c16b, in0=c16, scalar1=0.001, scalar2=None, op0=ALU.max
            )
            l1 = small.tile([P, 2 * G], F32, tag="l1")
            nc.scalar.activation(out=l1, in_=c16b, func=ACT_F.Ln)
            l2 = small.tile([P, 2 * G], F32, tag="l2")
            nc.scalar.activation(out=l2, in_=c16b, func=ACT_F.Ln, scale=-1.0, bias=1.0)
            rev = small.tile([P, 2 * G], F32, tag="rev")
            nc.vector.tensor_tensor(out=rev, in0=l1, in1=l2, op=ALU.subtract)
            q2 = small.tile([P, G], F32, tag="q2")
            nc.vector.tensor_scalar(
                out=q2, in0=rev[:, G : 2 * G], scalar1=agw_w, scalar2=None,
                op0=ALU.mult,
            )
            logit = small.tile([P, G], F32, tag="logit")
            nc.vector.scalar_tensor_tensor(
                out=logit, in0=rev[:, 0:G], scalar=gw, in1=q2,
                op0=ALU.mult, op1=ALU.add,
            )
            se = small.tile([P, G], F32, tag="se")
            nc.scalar.activation(out=se, in_=logit, func=ACT_F.Exp, scale=-1.0)
            sl8 = small.tile([P, G], F32, tag="sl8")
            nc.scalar.activation(out=sl8, in_=se, func=ACT_F.Ln, bias=1.0)
            sg8 = small.tile([P, G], F32, tag="sg8")
            nc.scalar.activation(out=sg8, in_=sl8, func=ACT_F.Exp, scale=-1.0)
            g8 = small.tile([P, G], F32, tag="g8")
            nc.vector.tensor_tensor(
                out=g8, in0=sg8, in1=agw_sb[:, t0 : t0 + G], op=ALU.mult
            )
            nc.vector.tensor_copy(out=g_staged[:, t0 : t0 + G], in_=g8)

            for i in range(G):
                t = t0 + i
                o_sb = outp.tile([P, D], F32, tag="o")
                # out = pair * g  (ACT copy with per-partition scale)
                nc.scalar.activation(
                    out=o_sb, in_=p_l[i], func=ACT_F.Copy,
                    scale=g8[:, i : i + 1],
                )
                nc.sync.dma_start(out=dout[ts(t, P), :], in_=o_sb)

        nc.sync.dma_start(out=dg[:, :], in_=g_staged)

    nc.compile()
    return nc


_NC_CACHE = {}


def _get_nc(gw, agw_w, fast_main, fast_aux):
    key = (round(gw, 9), round(agw_w, 9), fast_main, fast_aux)
    if key not in _NC_CACHE:
        _NC_CACHE[key] = _build_nc(gw, agw_w, fast_main, fast_aux)
    return _NC_CACHE[key]


def kernel(
    unary_term,
    pair_term,
    auxiliary_term,
    auxiliary_gating_weight,
    ln_gamma,
    ln_beta,
    W,
    b,
    ln_aux_gamma,
    ln_aux_beta,
    W_aux,
    b_aux,
    gate_weight,
    aux_gate_weight,
):
    global last_results
    f32 = np.float32
    bf16 = ml_dtypes.bfloat16

    unary_term = np.ascontiguousarray(unary_term, dtype=f32)
    pair_term = np.ascontiguousarray(pair_term, dtype=f32)
    auxiliary_term = np.ascontiguousarray(auxiliary_term, dtype=f32)
    agw_in = np.ascontiguousarray(auxiliary_gating_weight, dtype=f32)
    ln_gamma = np.asarray(ln_gamma, dtype=f32)
    ln_beta = np.asarray(ln_beta, dtype=f32)
    W = np.asarray(W, dtype=f32)
    b = np.asarray(b, dtype=f32)
    ln_aux_gamma = np.asarray(ln_aux_gamma, dtype=f32)
    ln_aux_beta = np.asarray(ln_aux_beta, dtype=f32)
    W_aux = np.asarray(W_aux, dtype=f32)
    b_aux = np.asarray(b_aux, dtype=f32)
    gw = float(np.asarray(gate_weight))
    agw_w = float(np.asarray(aux_gate_weight))

    # Fast path requires beta==0 and gamma>0 (see module docstring).
    fast_main = bool(np.all(ln_beta == 0.0) and np.all(ln_gamma > 0.0))
    fast_aux = bool(np.all(ln_aux_beta == 0.0) and np.all(ln_aux_gamma > 0.0))

    if fast_main:
        W_eff = W * ln_gamma[:, None]
        b_eff = b.copy()
    else:
        W_eff = W
        b_eff = b + ln_beta @ W
    if fast_aux:
        Wa_eff = W_aux * ln_aux_gamma[:, None]
        ba_eff = b_aux.copy()
    else:
        Wa_eff = W_aux
        ba_eff = b_aux + ln_aux_beta @ W_aux

    # weight chunks: [128, KC, FD], partition = d-within-chunk
    wr = np.ascontiguousarray(
        W_eff.reshape(KC, P, FD).transpose(1, 0, 2).reshape(P, KC * FD).astype(bf16)
    )
    waux_r = np.ascontiguousarray(
        Wa_eff.reshape(KA, P, F8).transpose(1, 0, 2).reshape(P, KA * F8).astype(bf16)
    )
    b_row = np.concatenate([b_eff, ba_eff]).reshape(1, FT).astype(f32)

    nc = _get_nc(gw, agw_w, fast_main, fast_aux)

    in_maps = []
    for c in range(N_CORES):
        r0 = c * ROWS
        m = {
            "u": unary_term[r0 : r0 + ROWS],
            "p": pair_term[r0 : r0 + ROWS],
            "ax": auxiliary_term[r0 : r0 + ROWS],
            "agw_t": np.ascontiguousarray(
                agw_in[r0 : r0 + ROWS].reshape(NT, P).T
            ),
            "wr": wr,
            "waux_r": waux_r,
            "b_row": b_row,
        }
        if not fast_main:
            m["gam_row"] = np.ascontiguousarray(ln_gamma.reshape(1, D2))
            m["negbeta_row"] = np.ascontiguousarray(-ln_beta.reshape(1, D2))
        if not fast_aux:
            m["gam_aux_row"] = np.ascontiguousarray(ln_aux_gamma.reshape(1, A))
            m["negbeta_aux_row"] = np.ascontiguousarray(-ln_aux_beta.reshape(1, A))
        in_maps.append(m)

    trace = bool(int(os.environ.get("KERNEL_TRACE", "0")))
    res = run_bass_kernel_spmd(
        nc, in_maps, core_ids=list(range(N_CORES)), trace=trace
    )
    last_results = res

    out = np.concatenate([res.results[c]["out"] for c in range(N_CORES)], axis=0)
    g = np.concatenate(
        [res.results[c]["g_t"].T.reshape(ROWS) for c in range(N_CORES)], axis=0
    )
    return out.astype(f32), g.astype(f32)


# revision 20
# speedup vs baseline: 1.0966x; 1.0966x over previous
"""Trainium2 Bass kernel for MessagePassingUnitGatingWithRelnessLogits.

Computation (per row n of N=32768):
  paired = relu(concat(unary, pair))                       # [2048]
  h      = relu(LN(paired) * gamma + beta)                 # [2048]
  gate   = mean_j sigmoid(h @ W + b)                       # scalar
  ha     = relu(LN(aux) * gamma_a + beta_a)                # [256]
  agate  = mean_j sigmoid(ha @ W_aux + b_aux)              # scalar
  logit  = gw * rev_sigmoid(gate) + agw * rev_sigmoid(agate)
  g      = sigmoid(logit) * auxiliary_gating_weight
  out    = pair * g
Returns (out [N,1024] f32, g [N] f32).

Sharding: pure data-parallel over N across 8 cores (4096 rows each).

Architecture notes:
 - All transcendentals via exp/ln only -> single ACT table set, no swaps.
 - Fast path (beta==0, gamma>0): relu((x-mu)*rs*gamma) = rs * gamma*relu(x-mu),
   so gamma folds into W on the host and the per-row rs folds into the tiny
   [128,72] PSUM evacuation.  Normalize+relu is ONE dual-op DVE pass.
 - LN stats: row sums ride the ACT relu passes (accum_out); sum-of-squares is
   one DVE scalar_tensor_tensor (x*x) with accum_out.  bn_stats only for aux.
 - h transposed via one DMA xbar transpose per 2-tile block; emission is
   software-pipelined one group so transposes never stall fresh loads on the
   in-order Sync queue.
 - Gate matmul: lhsT = hT chunk (stationary), rhs = W chunk (moving),
   y lands row-major [128 rows, 72] in PSUM; MMs stream at ~53ns.
 - unary/aux shipped as bf16 from the host (they only feed the LN path).
 - aux loads + output stores issue from GPSIMD's SWDGE to offload Sync.
"""

import os
import sys

import numpy as np

if "/opt/trn_rl_repo" not in sys.path:
    sys.path.insert(0, "/opt/trn_rl_repo")

import ml_dtypes  # noqa: E402

import bass_rust as _bass_rust  # noqa: E402

import concourse.bacc as bacc  # noqa: E402
import concourse.bass as bass  # noqa: E402
import concourse.mybir as mybir  # noqa: E402
import concourse.tile as tile  # noqa: E402
from concourse.bass import ts  # noqa: E402
from concourse.bass_utils import run_bass_kernel_spmd  # noqa: E402
from concourse.hw_specs import get_activation_tables  # noqa: E402

F32 = mybir.dt.float32
BF16 = mybir.dt.bfloat16
ALU = mybir.AluOpType
ACT_F = mybir.ActivationFunctionType

N_CORES = 8
N = 32768
D = 1024
D2 = 2048
A = 256
FD = 64
F8 = 8
FT = FD + F8                 # 72 packed gate columns (main | aux)
EPS = 1e-5

ROWS = N // N_CORES          # 4096 rows per core
P = 128                      # partition tile
NT = ROWS // P               # 32 tiles per core
G = 4                        # tiles per group (batched scalar chains)
NG = NT // G                 # 8 groups
KC = D2 // P                 # 16 contraction chunks (main)
KA = A // P                  # 2 contraction chunks (aux)
KT = KC + KA                 # 18 chunks per tile in the fused transpose

ACT_SET = "natural_log_exp_and_others"

# module-level stash so test.py can read profiling info
last_results = None


class _Bacc(bacc.Bacc):
    """Bacc that pins every ACTIVATE to one activation-table set.

    The default chooser picks the first set containing each function,
    which thrashes between sets (~2.7us per swap).  Every function this
    kernel uses (relu, exp, ln, copy, identity) lives in
    `natural_log_exp_and_others`, so blank out the other sets (keeping
    list positions, which are the set ids).
    """

    def insert_act_table_loads(self):
        has_activation = any(
            isinstance(i, mybir.InstActivation)
            for b in self.main_func.blocks
            for i in b.instructions
        )
        if not has_activation:
            return
        tables = [
            (name, (funcs if name == ACT_SET else set()))
            for name, funcs in get_activation_tables(self.m.arch).items()
        ]
        _bass_rust.insert_act_table_loads(self, tables)


def _build_nc(gw: float, agw_w: float, fast_main: bool, fast_aux: bool):
    nc = _Bacc("TRN2", target_bir_lowering=False, debug=False, num_devices=N_CORES)

    # ---------------- DRAM I/O ----------------
    du = nc.dram_tensor("u", [ROWS, D], BF16, kind="ExternalInput")
    dp = nc.dram_tensor("p", [ROWS, D], F32, kind="ExternalInput")
    dax = nc.dram_tensor("ax", [ROWS, A], BF16, kind="ExternalInput")
    dagw = nc.dram_tensor("agw_t", [P, NT], F32, kind="ExternalInput")
    dwr = nc.dram_tensor("wr", [P, KC * FD], BF16, kind="ExternalInput")
    dwaux = nc.dram_tensor("waux_r", [P, KA * F8], BF16, kind="ExternalInput")
    db = nc.dram_tensor("b_row", [1, FT], F32, kind="ExternalInput")
    dgb = dnbb = dgba = dnbba = None
    if not fast_main:
        dgb = nc.dram_tensor("gam_row", [1, D2], F32, kind="ExternalInput")
        dnbb = nc.dram_tensor("negbeta_row", [1, D2], F32, kind="ExternalInput")
    if not fast_aux:
        dgba = nc.dram_tensor("gam_aux_row", [1, A], F32, kind="ExternalInput")
        dnbba = nc.dram_tensor("negbeta_aux_row", [1, A], F32, kind="ExternalInput")

    dout = nc.dram_tensor("out", [ROWS, D], F32, kind="ExternalOutput")
    dg = nc.dram_tensor("g_t", [P, NT], F32, kind="ExternalOutput")

    import contextlib

    ctx = contextlib.ExitStack()
    with ctx:
        tc = ctx.enter_context(tile.TileContext(nc))

        consts = ctx.enter_context(tc.tile_pool(name="consts", bufs=1))
        io_u = ctx.enter_context(tc.tile_pool(name="io_u", bufs=3))
        io_p = ctx.enter_context(tc.tile_pool(name="io_p", bufs=6))
        io_ax = ctx.enter_context(tc.tile_pool(name="io_ax", bufs=6))
        prd = ctx.enter_context(tc.tile_pool(name="prd", bufs=2 * G + 2))
        sqp = ctx.enter_context(tc.tile_pool(name="sqp", bufs=2))
        hp = ctx.enter_context(tc.tile_pool(name="hp", bufs=2))
        htp = ctx.enter_context(tc.tile_pool(name="htp", bufs=2))
        stg = ctx.enter_context(tc.tile_pool(name="stg", bufs=2))
        outp = ctx.enter_context(tc.tile_pool(name="outp", bufs=3))
        small = ctx.enter_context(tc.tile_pool(name="small", bufs=3))
        ps_y = ctx.enter_context(tc.tile_pool(name="ps_y", bufs=6, space="PSUM"))

        # ---------------- constants ----------------
        wr_sb = consts.tile([P, KC, FD], BF16)
        nc.sync.dma_start(out=wr_sb, in_=dwr[:, :].rearrange("p (k f) -> p k f", k=KC))
        waux_sb = consts.tile([P, KA, F8], BF16)
        nc.sync.dma_start(
            out=waux_sb, in_=dwaux[:, :].rearrange("p (k f) -> p k f", k=KA)
        )
        agw_sb = consts.tile([P, NT], F32)
        nc.sync.dma_start(out=agw_sb, in_=dagw[:, :])

        def bcast_load(dram_ap, width, dt=F32):
            t = consts.tile([P, width], dt)
            src = bass.AP(tensor=dram_ap.tensor, offset=0, ap=[[0, P], [1, width]])
            nc.gpsimd.dma_start(out=t, in_=src)
            return t

        b_bc = bcast_load(db[:, :], FT)
        gb_b = nbb_b = gba_b = nbba_b = None
        if not fast_main:
            gb_b = bcast_load(dgb[:, :], D2, BF16)
            nbb_b = bcast_load(dnbb[:, :], D2, BF16)
        if not fast_aux:
            gba_b = bcast_load(dgba[:, :], A, BF16)
            nbba_b = bcast_load(dnbba[:, :], A, BF16)

        eps_col = consts.tile([P, 1], F32)
        nc.vector.memset(eps_col, EPS)
        g_staged = consts.tile([P, NT], F32)

        # per-group live state, keyed by group index
        st = {}

        def phase_a(grp):
            t0 = grp * G
            s_u = small.tile([P, G], F32, tag="s_u")
            s_p = small.tile([P, G], F32, tag="s_p")
            ssq = small.tile([P, G], F32, tag="ssq")
            mva = small.tile([P, G, 2], F32, tag="mva")
            u2_l, p2_l, ax2_l, paired_l = [], [], [], []
            for j in range(G // 2):
                tt = t0 + 2 * j
                u2 = io_u.tile([P, 2, D], BF16, tag="u2")
                p2 = io_p.tile([P, 2, D], F32, tag="p2")
                ax2 = io_ax.tile([P, 2, A], BF16, tag="ax2")
                nc.sync.dma_start(
                    out=u2, in_=du[ts(tt // 2, 2 * P), :].rearrange(
                        "(s p) c -> p s c", p=P
                    )
                )
                nc.sync.dma_start(
                    out=p2, in_=dp[ts(tt // 2, 2 * P), :].rearrange(
                        "(s p) c -> p s c", p=P
                    )
                )
                nc.gpsimd.dma_start(
                    out=ax2, in_=dax[ts(tt // 2, 2 * P), :].rearrange(
                        "(s p) c -> p s c", p=P
                    )
                )
                u2_l.append(u2)
                p2_l.append(p2)
                ax2_l.append(ax2)
            for i in range(G):
                j, s = divmod(i, 2)
                paired = prd.tile([P, D2], BF16, tag="paired")
                nc.scalar.activation(
                    out=paired[:, 0:D], in_=u2_l[j][:, s, :], func=ACT_F.Relu,
                    accum_out=s_u[:, i : i + 1],
                )
                nc.scalar.activation(
                    out=paired[:, D:D2], in_=p2_l[j][:, s, :], func=ACT_F.Relu,
                    accum_out=s_p[:, i : i + 1],
                )
                sq = sqp.tile([P, D2], BF16, tag="sq")
                nc.vector.scalar_tensor_tensor(
                    out=sq, in0=paired, scalar=0.0, in1=paired,
                    op0=ALU.bypass, op1=ALU.mult, accum_out=ssq[:, i : i + 1],
                )
                bnsta = small.tile([P, 6], F32, tag="bnsta")
                nc.vector.bn_stats(out=bnsta, in_=ax2_l[j][:, s, :])
                nc.vector.bn_aggr(out=mva[:, i, :], in_=bnsta)
                paired_l.append(paired)
            st[grp] = dict(
                s_u=s_u, s_p=s_p, ssq=ssq, mva=mva,
                p2=p2_l, ax2=ax2_l, paired=paired_l,
            )

        def phase_s(grp):
            d = st[grp]
            tsum = small.tile([P, G], F32, tag="tsum")
            nc.vector.tensor_tensor(out=tsum, in0=d["s_u"], in1=d["s_p"], op=ALU.add)
            negmu = small.tile([P, G], F32, tag="negmu")
            nc.vector.tensor_scalar(
                out=negmu, in0=tsum, scalar1=-1.0 / D2, scalar2=None, op0=ALU.mult
            )
            musq = small.tile([P, G], F32, tag="musq")
            nc.vector.tensor_tensor(out=musq, in0=negmu, in1=negmu, op=ALU.mult)
            var2 = small.tile([P, 2 * G], F32, tag="var2")
            nc.vector.scalar_tensor_tensor(
                out=var2[:, 0:G], in0=d["ssq"], scalar=1.0 / D2, in1=musq,
                op0=ALU.mult, op1=ALU.subtract,
            )
            nc.vector.tensor_copy(out=var2[:, G : 2 * G], in_=d["mva"][:, :, 1])
            negmua = small.tile([P, G], F32, tag="negmua")
            nc.vector.tensor_scalar(
                out=negmua, in0=d["mva"][:, :, 0], scalar1=-1.0, scalar2=None,
                op0=ALU.mult,
            )
            lnv = small.tile([P, 2 * G], F32, tag="lnv")
            nc.scalar.activation(out=lnv, in_=var2, func=ACT_F.Ln, bias=eps_col)
            rs2 = small.tile([P, 2 * G], F32, tag="rs2")
            nc.scalar.activation(out=rs2, in_=lnv, func=ACT_F.Exp, scale=-0.5)
            d["negmu"] = negmu
            d["negmua"] = negmua
            d["rs2"] = rs2

        def phase_b(grp):
            d = st[grp]
            staged = stg.tile([P, G * FT], BF16, tag="staged")
            for j in range(G // 2):
                hh2 = hp.tile([P, 2, KT * P], BF16, tag="hh2")
                for s in range(2):
                    i = 2 * j + s
                    if fast_main:
                        nc.vector.tensor_scalar(
                            out=hh2[:, s, 0:D2], in0=d["paired"][i],
                            scalar1=d["negmu"][:, i : i + 1], scalar2=0.0,
                            op0=ALU.add, op1=ALU.max,
                        )
                    else:
                        nc.vector.tensor_scalar(
                            out=hh2[:, s, 0:D2], in0=d["paired"][i],
                            scalar1=d["negmu"][:, i : i + 1],
                            scalar2=d["rs2"][:, i : i + 1],
                            op0=ALU.add, op1=ALU.mult,
                        )
                        nc.vector.scalar_tensor_tensor(
                            out=hh2[:, s, 0:D2], in0=hh2[:, s, 0:D2], scalar=0.0,
                            in1=gb_b, op0=ALU.bypass, op1=ALU.mult,
                        )
                        nc.vector.scalar_tensor_tensor(
                            out=hh2[:, s, 0:D2], in0=hh2[:, s, 0:D2], scalar=0.0,
                            in1=nbb_b, op0=ALU.bypass, op1=ALU.max,
                        )
                    if fast_aux:
                        nc.vector.tensor_scalar(
                            out=hh2[:, s, D2 : KT * P], in0=d["ax2"][j][:, s, :],
                            scalar1=d["negmua"][:, i : i + 1], scalar2=0.0,
                            op0=ALU.add, op1=ALU.max,
                        )
                    else:
                        nc.vector.tensor_scalar(
                            out=hh2[:, s, D2 : KT * P], in0=d["ax2"][j][:, s, :],
                            scalar1=d["negmua"][:, i : i + 1],
                            scalar2=d["rs2"][:, G + i : G + i + 1],
                            op0=ALU.add, op1=ALU.mult,
                        )
                        nc.vector.scalar_tensor_tensor(
                            out=hh2[:, s, D2 : KT * P],
                            in0=hh2[:, s, D2 : KT * P], scalar=0.0,
                            in1=gba_b, op0=ALU.bypass, op1=ALU.mult,
                        )
                        nc.vector.scalar_tensor_tensor(
                            out=hh2[:, s, D2 : KT * P],
                            in0=hh2[:, s, D2 : KT * P], scalar=0.0,
                            in1=nbba_b, op0=ALU.bypass, op1=ALU.max,
                        )
                # one xbar transpose for 2 tiles: [128, 4608] -> [128, 36, 128]
                hht2 = htp.tile([P, 2 * KT, P], BF16, tag="hht2")
                nc.sync.dma_start_transpose(
                    hht2, hh2[:, :, :].rearrange("p s c -> p (s c)")
                )
                for s in range(2):
                    i = 2 * j + s
                    off = s * KT
                    yps = ps_y.tile([P, FT], F32, tag="yps")
                    for k in range(KC):
                        nc.tensor.matmul(
                            yps[:, 0:FD], lhsT=hht2[:, off + k, :],
                            rhs=wr_sb[:, k, :],
                            start=(k == 0), stop=(k == KC - 1),
                        )
                    for k in range(KA):
                        nc.tensor.matmul(
                            yps[:, FD:FT], lhsT=hht2[:, off + KC + k, :],
                            rhs=waux_sb[:, k, :],
                            start=(k == 0), stop=(k == KA - 1),
                        )
                    sl = staged[:, i * FT : i * FT + FD]
                    sla = staged[:, i * FT + FD : (i + 1) * FT]
                    if fast_main:
                        nc.vector.scalar_tensor_tensor(
                            out=sl, in0=yps[:, 0:FD],
                            scalar=d["rs2"][:, i : i + 1],
                            in1=b_bc[:, 0:FD], op0=ALU.mult, op1=ALU.add,
                        )
                    else:
                        nc.vector.scalar_tensor_tensor(
                            out=sl, in0=yps[:, 0:FD], scalar=0.0,
                            in1=b_bc[:, 0:FD], op0=ALU.bypass, op1=ALU.add,
                        )
                    if fast_aux:
                        nc.vector.scalar_tensor_tensor(
                            out=sla, in0=yps[:, FD:FT],
                            scalar=d["rs2"][:, G + i : G + i + 1],
                            in1=b_bc[:, FD:FT], op0=ALU.mult, op1=ALU.add,
                        )
                    else:
                        nc.vector.scalar_tensor_tensor(
                            out=sla, in0=yps[:, FD:FT], scalar=0.0,
                            in1=b_bc[:, FD:FT], op0=ALU.bypass, op1=ALU.add,
                        )
            d["staged"] = staged

        def phase_cd(grp):
            t0 = grp * G
            d = st[grp]
            staged = d["staged"]
            # sigma(z) = exp(-ln(1 + exp(-z)))
            ebuf = stg.tile([P, G * FT], BF16, tag="ebuf")
            nc.scalar.activation(out=ebuf, in_=staged, func=ACT_F.Exp, scale=-1.0)
            lbuf = stg.tile([P, G * FT], BF16, tag="lbuf")
            nc.scalar.activation(out=lbuf, in_=ebuf, func=ACT_F.Ln, bias=1.0)
            sig = stg.tile([P, G * FT], BF16, tag="sig")
            nc.scalar.activation(out=sig, in_=lbuf, func=ACT_F.Exp, scale=-1.0)

            sig3 = sig[:, :].rearrange("p (g f) -> p g f", g=G)
            m4 = small.tile([P, G], F32, tag="m4")
            nc.vector.tensor_reduce(
                out=m4, in_=sig3[:, :, 0:FD], axis=mybir.AxisListType.X, op=ALU.add
            )
            ma4 = small.tile([P, G], F32, tag="ma4")
            nc.vector.tensor_reduce(
                out=ma4, in_=sig3[:, :, FD:FT], axis=mybir.AxisListType.X, op=ALU.add
            )
            c2 = small.tile([P, 2 * G], F32, tag="c2")
            nc.vector.tensor_scalar(
                out=c2[:, 0:G], in0=m4, scalar1=1.0 / FD, scalar2=0.999,
                op0=ALU.mult, op1=ALU.min,
            )
            nc.vector.tensor_scalar(
                out=c2[:, G : 2 * G], in0=ma4, scalar1=1.0 / F8, scalar2=0.999,
                op0=ALU.mult, op1=ALU.min,
            )
            c2b = small.tile([P, 2 * G], F32, tag="c2b")
            nc.vector.tensor_scalar(
                out=c2b, in0=c2, scalar1=0.001, scalar2=None, op0=ALU.max
            )
            l1 = small.tile([P, 2 * G], F32, tag="l1")
            nc.scalar.activation(out=l1, in_=c2b, func=ACT_F.Ln)
            l2 = small.tile([P, 2 * G], F32, tag="l2")
            nc.scalar.activation(out=l2, in_=c2b, func=ACT_F.Ln, scale=-1.0, bias=1.0)
            rev = small.tile([P, 2 * G], F32, tag="rev")
            nc.vector.tensor_tensor(out=rev, in0=l1, in1=l2, op=ALU.subtract)
            q2 = small.tile([P, G], F32, tag="q2")
            nc.vector.tensor_scalar(
                out=q2, in0=rev[:, G : 2 * G], scalar1=agw_w, scalar2=None,
                op0=ALU.mult,
            )
            logit = small.tile([P, G], F32, tag="logit")
            nc.vector.scalar_tensor_tensor(
                out=logit, in0=rev[:, 0:G], scalar=gw, in1=q2,
                op0=ALU.mult, op1=ALU.add,
            )
            se = small.tile([P, G], F32, tag="se")
            nc.scalar.activation(out=se, in_=logit, func=ACT_F.Exp, scale=-1.0)
            sl8 = small.tile([P, G], F32, tag="sl8")
            nc.scalar.activation(out=sl8, in_=se, func=ACT_F.Ln, bias=1.0)
            sg8 = small.tile([P, G], F32, tag="sg8")
            nc.scalar.activation(out=sg8, in_=sl8, func=ACT_F.Exp, scale=-1.0)
            g8 = small.tile([P, G], F32, tag="g8")
            nc.vector.tensor_tensor(
                out=g8, in0=sg8, in1=agw_sb[:, t0 : t0 + G], op=ALU.mult
            )
            nc.vector.tensor_copy(out=g_staged[:, t0 : t0 + G], in_=g8)

            for j in range(G // 2):
                tt = t0 + 2 * j
                o2 = outp.tile([P, 2, D], F32, tag="o2")
                for s in range(2):
                    i = 2 * j + s
                    if j % 2 == 0:
                        # ACT: out = pair * g (copy with per-partition scale)
                        nc.scalar.activation(
                            out=o2[:, s, :], in_=d["p2"][j][:, s, :],
                            func=ACT_F.Copy, scale=g8[:, i : i + 1],
                        )
                    else:
                        nc.vector.tensor_scalar(
                            out=o2[:, s, :], in0=d["p2"][j][:, s, :],
                            scalar1=g8[:, i : i + 1], scalar2=None, op0=ALU.mult,
                        )
                nc.gpsimd.dma_start(
                    out=dout[ts(tt // 2, 2 * P), :].rearrange(
                        "(s p) c -> p s c", p=P
                    ),
                    in_=o2,
                )
            del st[grp]

        # ---------------- software-pipelined main loop ----------------
        for grp in range(NG):
            phase_a(grp)
            if grp >= 1:
                phase_s(grp - 1)
                phase_b(grp - 1)
                phase_cd(grp - 1)
        phase_s(NG - 1)
        phase_b(NG - 1)
        phase_cd(NG - 1)

        nc.sync.dma_start(out=dg[:, :], in_=g_staged)

    nc.compile()
    return nc


_NC_CACHE = {}


def _get_nc(gw, agw_w, fast_main, fast_aux):
    key = (round(gw, 9), round(agw_w, 9), fast_main, fast_aux)
    if key not in _NC_CACHE:
        _NC_CACHE[key] = _build_nc(gw, agw_w, fast_main, fast_aux)
    return _NC_CACHE[key]


def kernel(
    unary_term,
    pair_term,
    auxiliary_term,
    auxiliary_gating_weight,
    ln_gamma,
    ln_beta,
    W,
    b,
    ln_aux_gamma,
    ln_aux_beta,
    W_aux,
    b_aux,
    gate_weight,
    aux_gate_weight,
):
    global last_results
    f32 = np.float32
    bf16 = ml_dtypes.bfloat16

    unary_bf = np.asarray(unary_term).astype(bf16)
    pair_term = np.ascontiguousarray(pair_term, dtype=f32)
    aux_bf = np.asarray(auxiliary_term).astype(bf16)
    agw_in = np.ascontiguousarray(auxiliary_gating_weight, dtype=f32)
    ln_gamma = np.asarray(ln_gamma, dtype=f32)
    ln_beta = np.asarray(ln_beta, dtype=f32)
    W = np.asarray(W, dtype=f32)
    b = np.asarray(b, dtype=f32)
    ln_aux_gamma = np.asarray(ln_aux_gamma, dtype=f32)
    ln_aux_beta = np.asarray(ln_aux_beta, dtype=f32)
    W_aux = np.asarray(W_aux, dtype=f32)
    b_aux = np.asarray(b_aux, dtype=f32)
    gw = float(np.asarray(gate_weight))
    agw_w = float(np.asarray(aux_gate_weight))

    # Fast path requires beta==0 and gamma>0 (see module docstring).
    fast_main = bool(np.all(ln_beta == 0.0) and np.all(ln_gamma > 0.0))
    fast_aux = bool(np.all(ln_aux_beta == 0.0) and np.all(ln_aux_gamma > 0.0))

    if fast_main:
        W_eff = W * ln_gamma[:, None]
        b_eff = b.copy()
    else:
        W_eff = W
        b_eff = b + ln_beta @ W
    if fast_aux:
        Wa_eff = W_aux * ln_aux_gamma[:, None]
        ba_eff = b_aux.copy()
    else:
        Wa_eff = W_aux
        ba_eff = b_aux + ln_aux_beta @ W_aux

    # weight chunks: [128, KC, FD], partition = d-within-chunk
    wr = np.ascontiguousarray(
        W_eff.reshape(KC, P, FD).transpose(1, 0, 2).reshape(P, KC * FD).astype(bf16)
    )
    waux_r = np.ascontiguousarray(
        Wa_eff.reshape(KA, P, F8).transpose(1, 0, 2).reshape(P, KA * F8).astype(bf16)
    )
    b_row = np.concatenate([b_eff, ba_eff]).reshape(1, FT).astype(f32)

    nc = _get_nc(gw, agw_w, fast_main, fast_aux)

    in_maps = []
    for c in range(N_CORES):
        r0 = c * ROWS
        m = {
            "u": unary_bf[r0 : r0 + ROWS],
            "p": pair_term[r0 : r0 + ROWS],
            "ax": aux_bf[r0 : r0 + ROWS],
            "agw_t": np.ascontiguousarray(
                agw_in[r0 : r0 + ROWS].reshape(NT, P).T
            ),
            "wr": wr,
            "waux_r": waux_r,
            "b_row": b_row,
        }
        if not fast_main:
            m["gam_row"] = np.ascontiguousarray(ln_gamma.reshape(1, D2))
            m["negbeta_row"] = np.ascontiguousarray(-ln_beta.reshape(1, D2))
        if not fast_aux:
            m["gam_aux_row"] = np.ascontiguousarray(ln_aux_gamma.reshape(1, A))
            m["negbeta_aux_row"] = np.ascontiguousarray(-ln_aux_beta.reshape(1, A))
        in_maps.append(m)

    trace = bool(int(os.environ.get("KERNEL_TRACE", "0")))
    res = run_bass_kernel_spmd(
        nc, in_maps, core_ids=list(range(N_CORES)), trace=trace
    )
    last_results = res

    out = np.concatenate([res.results[c]["out"] for c in range(N_CORES)], axis=0)
    g = np.concatenate(
        [res.results[c]["g_t"].T.reshape(ROWS) for c in range(N_CORES)], axis=0
    )
    return out.astype(f32), g.astype(f32)
